# revision 1
# baseline (speedup 1.0000x reference)
"""Trainium2 Bass kernel for multi-head causal attention + output projection.

Problem (hardcoded): B=4, T=2048, E=1024, H=16, D=64, float32.
  q = einsum('bte,hed->bhtd', data, Wq)   (same k, v)
  scores = q@k.T / sqrt(D), causal mask, softmax
  out = (attn @ v) concat-heads @ Wp + bp

Sharding across 8 NeuronCores: core c -> (batch b=c//2, head-group g=c%2).
Each core computes 8 heads of one batch:
  - QKV projections from x.T (E-major layout, fed transposed from host)
  - attention with scores kept TRANSPOSED (scoresT[tk, tq]) so softmax's
    cross-key reduction is done by the TensorEngine: V is augmented with a
    ones-column so attn@V also yields sum(exp) as output row 64.
  - no max-subtraction in softmax (scores ~ N(0,1) after the 1/8 scale;
    exp cannot overflow f32)
  - causal masking: whole key-tiles above the diagonal are skipped; the 4
    diagonal tiles per query-block get an additive -1e30 mask
  - partial output projection with this core's 512-row slice of Wp
Host side: out[b] = core(2b) + core(2b+1) partials, + bias.

Matmuls run as float32r (full-rate fp32 mode, 1 cycle/row for moving dim
>= 256); softmax weights and V are bf16 for the attn@V matmul.
"""

import numpy as np

import concourse.bass as bass
import concourse.mybir as mybir
import concourse.tile as tile
from contextlib import ExitStack

F32 = mybir.dt.float32
F32R = mybir.dt.float32r
BF16 = mybir.dt.bfloat16

NEG = -1.0e30

# Full-problem constants
B, T, E, H, D = 4, 2048, 1024, 16, 64
N_CORES = 8
H_LOC = H // 2          # heads per core
HP = H_LOC // 2         # head pairs per core
SCALE = float(D) ** -0.5

# Tunables
USE_F32R = True         # bitcast f32 operands to float32r for matmuls
EXP_BF16 = True         # softmax weights in bf16
CAUSAL = True           # skip fully-masked key tiles
AV_LAG = 5              # software pipeline depth between exp and attn@V
FLUSH_T = 4             # t-step at which the previous block's norm flushes


MMDT = F32R if USE_F32R else F32


def _mm_dt(ap):
    # operands reach matmuls already typed as MMDT
    return ap


def build_program(nc, *, T=T, E=E, HP=HP, causal=CAUSAL):
    """Emit the whole per-core program into `nc`. Shapes parameterized only
    so a scaled-down version can be validated in CoreSim.

    Emission order (PE executes in order; ACT/DVE hang off it):
      v, q(all pairs), k(0), k(1), attn(0), k(2), attn(1), k(3), attn(2),
      attn(3) [+ projection per query-block inside the last pair's attn]
    so the PE always has dense independent work while ACT runs exp, and
    pools are closed (ExitStack) as their tensors die to fit SBUF.
    """
    HL = 2 * HP                    # local heads
    C = HL * D                     # local concat width (512 full-size)
    ET = E // 128                  # e (embedding) 128-tiles
    TT = T // 128                  # token 128-tiles
    TQB = 512                      # query-block width
    NJB = T // TQB                 # query blocks
    DIAG = TQB // 128              # diagonal key-tiles per query block
    EW = min(512, E)               # projection output block width
    NE = E // EW
    CT = C // 128                  # concat 128-tiles (== HP)
    W2 = 2 * TQB                   # 2-bank psum slot width
    assert CT == HP and C <= TQB
    exp_dt = BF16 if EXP_BF16 else MMDT

    AL = mybir.AluOpType
    AF = mybir.ActivationFunctionType

    xTd = nc.dram_tensor("xT", [E, T], MMDT, kind="ExternalInput").ap()
    wqd = nc.dram_tensor("wq", [E, C], MMDT, kind="ExternalInput").ap()
    wkd = nc.dram_tensor("wk", [E, C], MMDT, kind="ExternalInput").ap()
    wvd = nc.dram_tensor("wv", [E, C], MMDT, kind="ExternalInput").ap()
    wpd = nc.dram_tensor("wp", [C, E], exp_dt, kind="ExternalInput").ap()
    # mask o duplicated into both halves: [mask_o | mask_o], o = 0..DIAG-1
    masks = nc.dram_tensor("masks", [128, DIAG * W2], exp_dt,
                           kind="ExternalInput").ap()
    out = nc.dram_tensor("out", [T, E], F32, kind="ExternalOutput").ap()

    with tile.TileContext(nc) as tc, ExitStack() as ctx:
        const = ctx.enter_context(tc.tile_pool(name="const", bufs=1))
        qk_pool = ctx.enter_context(tc.tile_pool(name="qk", bufs=2 * HP))
        vaug_pool = ctx.enter_context(tc.tile_pool(name="vaug", bufs=HL))
        # all PSUM through one pool of 4 two-bank slots
        psum = ctx.enter_context(tc.tile_pool(name="ps", bufs=4, space="PSUM"))

        mask_sb = const.tile([128, DIAG * W2], exp_dt, name="mask_sb")
        ones_sb = const.tile([1, 64], exp_dt, name="ones_sb")
        nc.vector.memset(ones_sb[:], 1.0)
        # ACT-touch scratch (see norm tail)
        tch = const.tile([1, 2], F32, name="tch")


        qT = [qk_pool.tile([128, T], MMDT, tag="qk", name=f"qT{i}")
              for i in range(HP)]
        kT = [qk_pool.tile([128, T], MMDT, tag="qk", name=f"kT{i}")
              for i in range(HP)]
        vaug = [vaug_pool.tile([128, TT * 65], exp_dt, tag="vaug",
                               name=f"vaug{i}") for i in range(HL)]

        def ps_tile(parts, name):
            return psum.tile([parts, W2], F32, tag="ps", name=name,
                             padded_shape=[128, W2])

        # input pools, manually released LIFO as their tensors die:
        # stack order (bottom->top): wk, xt, wq, wv
        wk_pool = tc.alloc_tile_pool(name="wkt", bufs=ET)
        xt_pool = tc.alloc_tile_pool(name="xt", bufs=ET)
        wq_pool = tc.alloc_tile_pool(name="wqt", bufs=ET)
        wv_pool = tc.alloc_tile_pool(name="wvt", bufs=ET)

        # ---------------- inputs (DMA order = consumption order) -----------
        xt, wqt, wkt, wvt = [], [], [], []
        for e in range(ET):
            wt = wv_pool.tile([128, C], MMDT, tag="wv", name=f"wvt{e}")
            nc.sync.dma_start(wt[:], wvd[e * 128:(e + 1) * 128, :])
            wvt.append(wt)
            xe = xt_pool.tile([128, T], MMDT, tag="xt", name=f"xt{e}")
            nc.sync.dma_start(xe[:], xTd[e * 128:(e + 1) * 128, :])
            xt.append(xe)
        for e in range(ET):
            wt = wq_pool.tile([128, C], MMDT, tag="wq", name=f"wqt{e}")
            nc.sync.dma_start(wt[:], wqd[e * 128:(e + 1) * 128, :])
            wqt.append(wt)
        for e in range(ET):
            wt = wk_pool.tile([128, C], MMDT, tag="wk", name=f"wkt{e}")
            nc.sync.dma_start(wt[:], wkd[e * 128:(e + 1) * 128, :])
            wkt.append(wt)
        nc.sync.dma_start(mask_sb[:], masks)

        # ---------------- v (natural [t, d] layout, e-outer groups) --------
        for h in range(HL):
            nc.vector.memset(vaug[h][:], 1.0)
        VG = 2 if TT % 2 == 0 else TT

        def emit_v_group(tg):
            psv = [ps_tile(128, f"psv{i}") for i in range(VG // 2)]
            for e in range(ET):
                for ti in range(VG):
                    t = tg + ti
                    nc.tensor.matmul(
                        psv[ti // 2][:, (ti % 2) * TQB:(ti % 2) * TQB + C],
                        _mm_dt(xt[e][:, t * 128:(t + 1) * 128]),
                        _mm_dt(wvt[e][:]),
                        start=(e == 0), stop=(e == ET - 1))
            for ti in range(VG):
                t = tg + ti
                for h in range(HL):
                    eng = nc.vector.tensor_copy if h % 2 == 0 else \
                        nc.scalar.copy
                    eng(vaug[h][:, t * 65:t * 65 + 64],
                        psv[ti // 2][:, (ti % 2) * TQB + h * 64:
                                     (ti % 2) * TQB + (h + 1) * 64])

        # ---------------- q/k projections ([d, t] layout, pairs packed) ----
        def emit_qk(p, wlist, dst, only_jbp=None):
            for jbp in range(0, NJB, 2):
                if only_jbp is not None and jbp != only_jbp:
                    continue
                pq = ps_tile(128, "psqk")
                nhalf = min(2, NJB - jbp)
                for e in range(ET):
                    for j in range(nhalf):
                        jb = jbp + j
                        sl = slice(j * TQB, (j + 1) * TQB)
                        nc.tensor.matmul(
                            pq[:, sl],
                            _mm_dt(wlist[e][:, p * 128:(p + 1) * 128]),
                            _mm_dt(xt[e][:, jb * TQB:(jb + 1) * TQB]),
                            start=(e == 0), stop=(e == ET - 1))
                w = nhalf * TQB
                nc.vector.tensor_copy(
                    dst[p][:, jbp * TQB:jbp * TQB + w], pq[:, 0:w])

        # v groups interleaved with q projections: q matmuls fill the PE
        # stalls while v psum groups wait on their evictions.
        vgroups = list(range(0, TT, VG))
        qpairs = list(range(HP))
        emit_v_group(vgroups[0])
        for i, tg in enumerate(vgroups[1:]):
            emit_v_group(tg)
            if i < len(qpairs):
                emit_qk(qpairs[i], wqt, qT)
        for p in qpairs[max(0, len(vgroups) - 1):]:
            emit_qk(p, wqt, qT)
        wv_pool.release()
        wq_pool.release()

        # attention pools on the RIGHT side so input pools can release
        # underneath them while attention overlaps the k projections.
        n_early = max(0, HP - 2)
        olt = [None] * CT
        olt_early = None
        if n_early:
            olt_early = tc.alloc_tile_pool(name="olt01", bufs=n_early,
                                           side="right")
            for i in range(n_early):
                olt[i] = olt_early.tile([128, T], exp_dt, tag="olt",
                                        name=f"olt{i}")
        exp_pool = tc.alloc_tile_pool(name="exp", bufs=7, side="right")
        sab_pool = tc.alloc_tile_pool(name="sab", bufs=2, side="right")

        norm_queue = []

        def emit_norm(p, jb, psAB):
            """Normalization + olt write for a finished (p, jb) block,
            deferred so it overlaps the next block's compute."""
            qsl = slice(jb * TQB, (jb + 1) * TQB)
            # evict out_av rows to SBUF; 1/sumexp straight to a bf16 row
            # (borrowed exp slot) feeding the broadcast matmul.
            sab = sab_pool.tile([64, W2], F32, tag="sab", name="sab")
            nc.vector.tensor_copy(sab[:], psAB[0:64, :])
            rr = exp_pool.tile([1, W2], exp_dt, tag="exp", name="rr")
            with nc.allow_low_precision(reason="softmax recip in bf16"):
                nc.vector.reciprocal(rr[:], psAB[64:65, :])
            psb = ps_tile(64, "psbc")
            nc.tensor.matmul(psb[:, 0:TQB], ones_sb[:], rr[:, 0:TQB],
                             start=True, stop=True)
            nc.tensor.matmul(psb[:, TQB:W2], ones_sb[:], rr[:, TQB:W2],
                             start=True, stop=True)
            for hh in range(2):
                nc.vector.scalar_tensor_tensor(
                    olt[p][64 * hh:64 * hh + 64, qsl],
                    sab[:, hh * TQB:(hh + 1) * TQB], 1.0,
                    psb[0:64, hh * TQB:(hh + 1) * TQB], AL.mult, AL.mult)
            # make ACT the slot's last reader: WAR waits then merge into the
            # ACT wait every PE matmul already carries (LW allows 1 wait)
            nc.scalar.copy(tch[:], psb[0:1, 0:2])
            if p == HP - 1:
                for it in range(DIAG * jb, DIAG * (jb + 1)):
                    emit_proj(it)

        def flush_norms():
            while norm_queue:
                emit_norm(*norm_queue.pop(0))

        def emit_attn_block(p, jb):
                n_tk = DIAG * (jb + 1) if causal else TT
                # heads A|B side by side: A cols 0:TQB, B cols TQB:2TQB
                psAB = ps_tile(65, "psAB")
                qsl = slice(jb * TQB, (jb + 1) * TQB)
                pend = []

                def flush_av(psAB=psAB, p=p, n_tk=n_tk, pend=pend):
                    t, ee = pend.pop(0)
                    last = t == n_tk - 1
                    nc.tensor.matmul(
                        psAB[:, 0:TQB],
                        _bf(vaug[2 * p][:, t * 65:t * 65 + 65]),
                        _bf(ee[:, 0:TQB]), start=(t == 0), stop=last)
                    nc.tensor.matmul(
                        psAB[:, TQB:W2],
                        _bf(vaug[2 * p + 1][:, t * 65:t * 65 + 65]),
                        _bf(ee[:, TQB:W2]), start=(t == 0), stop=last)

                for t in range(n_tk):
                    ksl = slice(t * 128, (t + 1) * 128)
                    psS = ps_tile(128, "psS")
                    nc.tensor.matmul(psS[:, 0:TQB],
                                     _mm_dt(kT[p][0:64, ksl]),
                                     _mm_dt(qT[p][0:64, qsl]),
                                     start=True, stop=True)
                    nc.tensor.matmul(psS[:, TQB:W2],
                                     _mm_dt(kT[p][64:128, ksl]),
                                     _mm_dt(qT[p][64:128, qsl]),
                                     start=True, stop=True)
                    ee = exp_pool.tile([128, W2], exp_dt, tag="exp",
                                       name="ee")
                    nc.scalar.activation(ee[:], psS[:], AF.Exp, scale=SCALE)
                    o = t - DIAG * jb
                    if o >= 0:
                        # zero above-diagonal entries (bf16 SBUF fast path)
                        nc.vector.tensor_mul(
                            ee[:], ee[:], mask_sb[:, o * W2:(o + 1) * W2])
                    pend.append((t, ee))
                    if t == FLUSH_T:
                        # previous block's deferred normalization, overlapped
                        # with this block's compute
                        flush_norms()
                    if len(pend) > AV_LAG:
                        flush_av()
                while pend:
                    flush_av()
                norm_queue.append((p, jb, psAB))

        def emit_proj(it):
            ps = ps_tile(128, "psp")
            for nb in range(NE):
                for c in range(CT):
                    nc.tensor.matmul(
                        ps[:, nb * EW:(nb + 1) * EW],
                        _bf(olt[c][:, it * 128:(it + 1) * 128]),
                        _bf(wpt[c][:, nb * EW:(nb + 1) * EW]),
                        start=(c == 0), stop=(c == CT - 1))
            ot = po_pool.tile([128, E], F32, tag="po", name="po")
            nc.vector.tensor_copy(ot[:], ps[:, 0:E])
            nc.sync.dma_start(out[it * 128:(it + 1) * 128, :], ot[:])

        # k projections interleaved with attention, one pair of lag
        for p in range(HP):
            emit_qk(p, wkt, kT)
            if 1 <= p <= HP - 2:
                for jb in range(NJB):
                    emit_attn_block(p - 1, jb)
        xt_pool.release()
        wk_pool.release()

        # late pools (space freed by xt/wk): remaining olt tiles, Wp, out
        olt_late = tc.alloc_tile_pool(name="olt23", bufs=CT - n_early,
                                      side="right")
        for i in range(n_early, CT):
            olt[i] = olt_late.tile([128, T], exp_dt, tag="olt",
                                   name=f"olt{i}")
        wp_pool = tc.alloc_tile_pool(name="wp", bufs=CT, side="right")
        po_pool = tc.alloc_tile_pool(name="po", bufs=3, side="right")
        wpt = []
        for c in range(CT):
            w = wp_pool.tile([128, E], exp_dt, tag="wp", name=f"wpt{c}")
            nc.sync.dma_start(w[:], wpd[c * 128:(c + 1) * 128, :])
            wpt.append(w)
        for p in range(max(0, HP - 2), HP):
            for jb in range(NJB):
                emit_attn_block(p, jb)
        flush_norms()
        # release right-side pools LIFO
        po_pool.release()
        wp_pool.release()
        olt_late.release()
        sab_pool.release()
        exp_pool.release()
        if olt_early is not None:
            olt_early.release()
    return nc


def _bf(ap):
    # attn@V operands are typed exp_dt (bf16 or MMDT) at allocation
    return ap


_WV_CACHE = {}


def wv_row(nc, wv_pool, wv, e, C):
    """Load (once) and return the e-th 128-row slice of Wv as an SBUF tile."""
    key = (id(nc), e)
    if key not in _WV_CACHE:
        wt = wv_pool.tile([128, C], MMDT, tag="wvt", name=f"wvt{e}")
        nc.sync.dma_start(wt[:], wv[e * 128:(e + 1) * 128, :])
        _WV_CACHE[key] = wt
    return _WV_CACHE[key][:]


def _wp_cast(a):
    if EXP_BF16:
        import ml_dtypes
        return a.astype(ml_dtypes.bfloat16)
    return a


def make_masks(diag=4, tqb=512):
    import ml_dtypes
    m = np.empty((128, diag * 2 * tqb), np.float32)
    p = np.arange(128)[:, None]
    f = np.arange(tqb)[None, :]
    for o in range(diag):
        blk = np.where(f >= p + 128 * o, 1.0, 0.0)
        m[:, o * 2 * tqb:o * 2 * tqb + tqb] = blk
        m[:, o * 2 * tqb + tqb:(o + 1) * 2 * tqb] = blk
    if EXP_BF16:
        return m.astype(ml_dtypes.bfloat16)
    return m


def shard_inputs(data, Wq, Wk, Wv, Wp):
    """Build the 8 per-core input maps from full inputs."""
    data = np.asarray(data, np.float32)
    Wq = np.asarray(Wq, np.float32)
    Wk = np.asarray(Wk, np.float32)
    Wv = np.asarray(Wv, np.float32)
    Wp = np.asarray(Wp, np.float32)
    masks = make_masks()
    in_maps = []
    for c in range(N_CORES):
        b, g = c // 2, c % 2
        hs = slice(g * H_LOC, (g + 1) * H_LOC)
        in_maps.append({
            "xT": np.ascontiguousarray(data[b].T),
            "wq": np.ascontiguousarray(
                Wq[hs].transpose(1, 0, 2).reshape(E, H_LOC * D)),
            "wk": np.ascontiguousarray(
                Wk[hs].transpose(1, 0, 2).reshape(E, H_LOC * D)),
            "wv": np.ascontiguousarray(
                Wv[hs].transpose(1, 0, 2).reshape(E, H_LOC * D)),
            "wp": _wp_cast(
                np.ascontiguousarray(Wp[g * H_LOC * D:(g + 1) * H_LOC * D, :])),
            "masks": masks,
        })
    return in_maps


_NC_CACHE = {}


def legalize_single_wait(nc):
    """This toolchain's walrus accepts at most ONE sync wait per engine
    instruction; Tile freely emits more. Split extra waits onto preceding
    same-engine NoOps (engine FIFOs make that equivalent)."""
    import bass_rust
    cnt = 0
    for f in nc.m.functions:
        for blk in f.blocks:
            new = []
            changed = False
            for inst in blk.instructions:
                si = inst.sync_info
                if si is not None and len(si.on_wait) > 1:
                    waits = list(si.on_wait)
                    for w in waits[:-1]:
                        nop = bass_rust.InstNoOp(name=f"legal_nop_{cnt}")
                        cnt += 1
                        nop.engine = inst.engine
                        nop.sync_info = bass_rust.SyncInfo(on_wait=[w],
                                                           on_update=[])
                        new.append(nop)
                    inst.sync_info = bass_rust.SyncInfo(
                        on_wait=[waits[-1]], on_update=list(si.on_update))
                    changed = True
                new.append(inst)
            if changed:
                blk.instructions = new
    return cnt


def get_nc():
    if "nc" not in _NC_CACHE:
        nc = bass.Bass("TRN2", target_bir_lowering=False, debug=False,
                       num_devices=N_CORES)
        build_program(nc)
        legalize_single_wait(nc)
        _NC_CACHE["nc"] = nc
    return _NC_CACHE["nc"]


def run(inputs, trace=False, **kw):
    """Run on the 8 NeuronCores; returns (full_output, BassKernelResults)."""
    from concourse.bass_utils import run_bass_kernel_spmd
    nc = get_nc()
    in_maps = shard_inputs(inputs["data"], inputs["Wq"], inputs["Wk"],
                           inputs["Wv"], inputs["Wp"])
    res = run_bass_kernel_spmd(nc, in_maps, core_ids=list(range(N_CORES)),
                               trace=trace, **kw)
    bp = np.asarray(inputs["bp"], np.float32)
    outf = np.empty((B, T, E), np.float32)
    for b in range(B):
        outf[b] = res.results[2 * b]["out"] + res.results[2 * b + 1]["out"] + bp
    return outf, res


def kernel(**inputs):
    out, _ = run(inputs)
    return out



# revision 8
# speedup vs baseline: 1.1480x; 1.1480x over previous
"""Trainium2 Bass kernel for multi-head causal attention + output projection.

Problem (hardcoded): B=4, T=2048, E=1024, H=16, D=64, float32.
  q = einsum('bte,hed->bhtd', data, Wq)   (same k, v)
  scores = q@k.T / sqrt(D), causal mask, softmax
  out = (attn @ v) concat-heads @ Wp + bp

Sharding across 8 NeuronCores: core c -> (batch b=c//2, head-group g=c%2).
Each core computes 8 heads (4 "pairs" of 2) of one batch.

Per-core design (all matmul operands bf16; PSUM f32):
 - q/k projections into qT/kT [d-pair(128) x T]; v into vallp[p]
   [t-in-tile(128) x (t-tile, head, d)] natural layout.
 - scores kept TRANSPOSED (psS[tk, tq]) so the softmax key-reduction is a
   PE contraction; exp on ACT (no max subtraction: scores ~ N(0,1)).
 - attn@V with the EXP TILE AS STATIONARY and V as moving: output lands
   in natural [tq, d] layout at full PE utilization (64-row moving), and
   sum(exp) comes from 1-row ones-matmuls into a shared psum bank.
 - normalization = per-partition reciprocal + tensor_scalar_mul on DVE
   (no PE broadcast needed in natural layout).
 - attention output transposed back to [c, t] via PE transpose for the
   output projection; Wp projection accumulates over 4 c-tiles.
 - causal: key-tiles above the diagonal skipped; diagonal tiles trim the
   query range to >= o*128 and mask only the 128x128 boundary triangle.
 - a filler queue spreads q/k/v chunk projections and the output
   projection into the ACT-bound attention windows so the PE never
   idles waiting for exp.

PSUM bank map (2KB x 8):
  banks 0-3: psS double-buffer (2 x [128,1024] f32)
  banks 4-5: psAV parity     (2 x [128,512] f32: 8 x 64-col accumulators)
  bank  6:   misc: sumexp accumulators (cols 0:16, parity 8) + transpose
             staging (bf16 view of f32 cols 64:320)
  bank  7:   pq: q/k/v chunk projections, then output projection

Host side: out[b] = core(2b) + core(2b+1) partials, + bias.
"""

import numpy as np

import concourse.bass as bass
import concourse.mybir as mybir
import concourse.tile as tile
from concourse import masks as cmasks
from contextlib import ExitStack

F32 = mybir.dt.float32
BF16 = mybir.dt.bfloat16

# Full-problem constants
B, T, E, H, D = 4, 2048, 1024, 16, 64
N_CORES = 8
H_LOC = H // 2          # heads per core
HP = H_LOC // 2         # head pairs per core
C = H_LOC * D           # local concat width (512)
ET = E // 128           # embedding 128-tiles
TT = T // 128           # token 128-tiles
NJB = 4                 # query blocks of 512
TQB = 512
SCALE = float(D) ** -0.5

AV_LAG = 5              # tiles between exp and attn@V consumption


def build_program(nc):
    AF = mybir.ActivationFunctionType

    xTd = nc.dram_tensor("xT", [E, T], BF16, kind="ExternalInput").ap()
    wqkvd = nc.dram_tensor("wqkv", [E, 3 * C], BF16,
                           kind="ExternalInput").ap()
    wpd = nc.dram_tensor("wp", [C, E], BF16, kind="ExternalInput").ap()
    maskd = nc.dram_tensor("mask", [128, 128], BF16,
                           kind="ExternalInput").ap()
    outd = nc.dram_tensor("out", [T, E], F32, kind="ExternalOutput").ap()

    with tile.TileContext(nc) as tc, ExitStack() as ctx:
        sb = ctx.enter_context(tc.tile_pool(name="sb", bufs=1))
        ident = sb.tile([128, 128], BF16, name="ident")
        mask_sb = sb.tile([128, 128], BF16, name="mask_sb")
        ones_mv = sb.tile([128, 1], BF16, name="ones_mv")
        wqkvt = [sb.tile([128, 3 * C], BF16, name=f"wqkvt{e}")
                 for e in range(ET)]
        xt = [sb.tile([128, T], BF16, name=f"xt{e}") for e in range(ET)]
        qT = [sb.tile([128, T], BF16, name=f"qT{p}") for p in range(HP)]
        kT = [sb.tile([128, T], BF16, name=f"kT{p}") for p in range(HP)]
        # v in natural layout, per pair: col = t*128 + h*64 + d
        vallp = [sb.tile([128, T], BF16, name=f"vallp{p}") for p in range(HP)]
        # attention out, natural [tq x (tt, h, d)] per pair
        olt_nat = [sb.tile([128, T], BF16, name=f"oltn{p}") for p in range(HP)]
        # attention out, transposed [c x t] per pair c-block
        olt = [sb.tile([128, T], BF16, name=f"olt{p}") for p in range(HP)]
        wpt = [sb.tile([128, E], BF16, name=f"wpt{p}") for p in range(HP)]

        ee_pool = ctx.enter_context(tc.tile_pool(name="ee", bufs=AV_LAG + 2))
        rr_pool = ctx.enter_context(tc.tile_pool(name="rr", bufs=4))
        ot_pool = ctx.enter_context(tc.tile_pool(name="ot", bufs=2))

        ps_pool = ctx.enter_context(
            tc.tile_pool(name="ps", bufs=1, space="PSUM"))
        psS_pool = ctx.enter_context(
            tc.tile_pool(name="pss", bufs=2, space="PSUM"))
        psAV_pool = ctx.enter_context(
            tc.tile_pool(name="psav", bufs=2, space="PSUM"))
        misc = ps_pool.tile([128, 512], F32, name="misc")
        pq = ps_pool.tile([128, 512], F32, name="pqbank")
        # transpose staging shares the pq bank (use is strictly sequential
        # with the projection matmuls); sumexp accumulators own misc.
        pq_bf = pq[:, 0:256].bitcast(BF16)  # [128, 512] bf16 region

        cmasks.make_identity(nc, ident[:])
        nc.vector.memset(ones_mv[:], 1.0)
        nc.sync.dma_start(mask_sb[:], maskd)
        for e in range(ET):
            nc.sync.dma_start(wqkvt[e][:], wqkvd[e * 128:(e + 1) * 128, :])
        for e in range(ET):
            nc.sync.dma_start(xt[e][:, 0:512], xTd[e * 128:(e + 1) * 128,
                                                   0:512])
        for cch in range(1, 4):
            for e in range(ET):
                nc.sync.dma_start(
                    xt[e][:, cch * 512:(cch + 1) * 512],
                    xTd[e * 128:(e + 1) * 128, cch * 512:(cch + 1) * 512])
        for p in range(HP):
            nc.sync.dma_start(wpt[p][:], wpd[p * 128:(p + 1) * 128, :])

        # ---------------- filler work-queue ------------------------------
        # Items: (key, fn). key=(p, c) for chunk work block (p, jb>=c)
        # depends on; None for output-projection work (no deadline).
        filler = []

        def emit_v_part(p, cch, half):
            # 2 token-tiles of chunk cch into pq regions (t%4)*128
            for ti in range(2):
                t = cch * 4 + half * 2 + ti
                r = (half * 2 + ti) * 128
                for e in range(ET):
                    nc.tensor.matmul(
                        pq[:, r:r + 128],
                        xt[e][:, t * 128:(t + 1) * 128],
                        wqkvt[e][:, 2 * C + p * 128:2 * C + (p + 1) * 128],
                        start=(e == 0), stop=(e == ET - 1))

        def emit_v_evict(p, cch):
            nc.vector.tensor_copy(
                vallp[p][:, cch * 512:(cch + 1) * 512], pq[:, 0:512])

        def emit_qk_part(p, cch, woff, half):
            # half of one 512-wide q or k chunk (4 e-steps)
            for e in range(4 * half, 4 * half + 4):
                nc.tensor.matmul(
                    pq[:, 0:512],
                    wqkvt[e][:, woff + p * 128:woff + (p + 1) * 128],
                    xt[e][:, cch * 512:(cch + 1) * 512],
                    start=(e == 0), stop=(e == ET - 1))

        def emit_qk_evict(dst, p, cch):
            nc.vector.tensor_copy(
                dst[p][:, cch * 512:(cch + 1) * 512], pq[:, 0:512])

        def enqueue_chunk(p, cch):
            key = (p, cch)
            filler.append((key, lambda: emit_v_part(p, cch, 0)))
            filler.append((key, lambda: (emit_v_part(p, cch, 1),
                                         emit_v_evict(p, cch))))
            filler.append((key, lambda: emit_qk_part(p, cch, 0, 0)))
            filler.append((key, lambda: (emit_qk_part(p, cch, 0, 1),
                                         emit_qk_evict(qT, p, cch))))
            filler.append((key, lambda: emit_qk_part(p, cch, C, 0)))
            filler.append((key, lambda: (emit_qk_part(p, cch, C, 1),
                                         emit_qk_evict(kT, p, cch))))

        def drain(n):
            for _ in range(n):
                if not filler:
                    return
                _, fn = filler.pop(0)
                fn()

        def drain_until(p, jb):
            while filler:
                key = filler[0][0]
                if key is not None:
                    kp, kc = key
                    if (kp, kc) > (p, jb):
                        return
                filler.pop(0)[1]()

        # ---------------- output projection work -------------------------
        def emit_transp(tt):
            for p4 in range(HP):
                nc.tensor.transpose(
                    pq_bf[:, p4 * 128:(p4 + 1) * 128],
                    olt_nat[p4][:, tt * 128:(tt + 1) * 128],
                    ident[:])

        def emit_transp_evict(tt):
            # GPSIMD cannot access PSUM; these evictions ride on DVE
            for p4 in range(HP):
                nc.vector.tensor_copy(
                    olt[p4][:, tt * 128:(tt + 1) * 128],
                    pq_bf[:, p4 * 128:(p4 + 1) * 128])

        def emit_proj_mm(tt, ec):
            for p4 in range(HP):
                nc.tensor.matmul(
                    pq[:, 0:512],
                    olt[p4][:, tt * 128:(tt + 1) * 128],
                    wpt[p4][:, ec * 512:(ec + 1) * 512],
                    start=(p4 == 0), stop=(p4 == HP - 1))

        def emit_proj_evict(tt, ec, ot):
            nc.vector.tensor_copy(ot[:, ec * 512:(ec + 1) * 512],
                                  pq[:, 0:512])

        def enqueue_proj(jb):
            for tt in range(4 * jb, 4 * jb + 4):
                ot = ot_pool.tile([128, E], F32, tag="ot", name="ot")

                def fin(tt=tt, ot=ot):
                    emit_proj_evict(tt, 1, ot)
                    nc.sync.dma_start(outd[tt * 128:(tt + 1) * 128, :],
                                      ot[:])
                filler.append((None, lambda tt=tt: emit_transp(tt)))
                filler.append((None, lambda tt=tt: emit_transp_evict(tt)))
                filler.append((None, lambda tt=tt: emit_proj_mm(tt, 0)))
                filler.append((None, lambda tt=tt, ot=ot: (
                    emit_proj_evict(tt, 0, ot), emit_proj_mm(tt, 1))))
                filler.append((None, fin))

        # ---------------- attention --------------------------------------
        def emit_attn_block(p, jb, blk):
            n_tk = 4 * jb + 4
            psAV = psAV_pool.tile([128, 512], F32, tag="av", name="psAV")
            sum_off = (blk % 2) * 8
            qsl_full = slice(jb * TQB, (jb + 1) * TQB)
            pend = []

            def flush_av():
                # start=True zeroes/marks the WHOLE 2KB psum bank: exactly
                # one start (first matmul of the block into the bank) and
                # one stop (last matmul) per bank per block; the pending-
                # zero mechanism zeroes each 64-col region at first write.
                tk, ee = pend.pop(0)
                o = tk - 4 * jb
                for h in range(2):
                    for s in range(max(0, o), 4):
                        st = ee[:, h * 512 + s * 128:h * 512 + (s + 1) * 128]
                        first = tk == 0 and h == 0 and s == max(0, o)
                        last = tk == n_tk - 1 and h == 1 and s == 3
                        nc.tensor.matmul(
                            psAV[:, (h * 4 + s) * 64:(h * 4 + s + 1) * 64],
                            st,
                            vallp[p][:, tk * 128 + h * 64:tk * 128 +
                                     (h + 1) * 64],
                            start=first, stop=last, skip_group_check=True)
                        nc.tensor.matmul(
                            misc[:, sum_off + h * 4 + s:
                                 sum_off + h * 4 + s + 1],
                            st, ones_mv[:],
                            start=first, stop=last, skip_group_check=True)

            for tk in range(n_tk):
                o = tk - 4 * jb
                lo = max(0, o) * 128
                ksl = slice(tk * 128, (tk + 1) * 128)
                psS = psS_pool.tile([128, 1024], F32, tag="s", name="psS")
                nc.tensor.matmul(
                    psS[:, lo:512],
                    kT[p][0:64, ksl],
                    qT[p][0:64, jb * TQB + lo:(jb + 1) * TQB],
                    start=True, stop=True)
                nc.tensor.matmul(
                    psS[:, 512 + lo:1024],
                    kT[p][64:128, ksl],
                    qT[p][64:128, jb * TQB + lo:(jb + 1) * TQB],
                    start=True, stop=True)
                ee = ee_pool.tile([128, 1024], BF16, tag="ee", name="ee")
                nc.scalar.activation(ee[:, lo:1024], psS[:, lo:1024],
                                     AF.Exp, scale=SCALE)
                if o >= 0:
                    for h in range(2):
                        r = slice(h * 512 + o * 128, h * 512 + (o + 1) * 128)
                        nc.vector.tensor_mul(ee[:, r], ee[:, r], mask_sb[:])
                pend.append((tk, ee))
                drain(2 if p == HP - 1 else 1)
                if len(pend) > AV_LAG:
                    flush_av()
            while pend:
                flush_av()

            # normalize: 1/sumexp per tq partition, scale psAV into olt_nat.
            # The reciprocal reads BOTH parity regions so the next block's
            # bank-wide start=True gets a WAR dependency on it (sound under
            # eager zero-region semantics).
            rr = rr_pool.tile([128, 16], F32, tag="rr", name="rr")
            nc.vector.reciprocal(rr[:], misc[:, 0:16])
            for h in range(2):
                for s in range(4):
                    tt = 4 * jb + s
                    nc.vector.tensor_scalar_mul(
                        olt_nat[p][:, tt * 128 + h * 64:tt * 128 +
                                   (h + 1) * 64],
                        psAV[:, (h * 4 + s) * 64:(h * 4 + s + 1) * 64],
                        rr[:, sum_off + h * 4 + s:sum_off + h * 4 + s + 1])

        # ---------------- schedule ---------------------------------------
        # prefix: chunk 0 of pair 0, emitted directly
        emit_v_part(0, 0, 0)
        emit_v_part(0, 0, 1)
        emit_v_evict(0, 0)
        emit_qk_part(0, 0, 0, 0)
        emit_qk_part(0, 0, 0, 1)
        emit_qk_evict(qT, 0, 0)
        emit_qk_part(0, 0, C, 0)
        emit_qk_part(0, 0, C, 1)
        emit_qk_evict(kT, 0, 0)
        # queue: remaining chunks in need order (pair-major)
        for p in range(HP):
            for cch in range(NJB):
                if p == 0 and cch == 0:
                    continue
                enqueue_chunk(p, cch)

        blk = 0
        for p in range(HP):
            for jb in range(NJB):
                drain_until(p, jb)
                emit_attn_block(p, jb, blk)
                blk += 1
                if p == HP - 1:
                    enqueue_proj(jb)
        drain(len(filler))
    return nc


# ---------------- host side ----------------------------------------------

def _bf(a):
    import ml_dtypes
    return np.ascontiguousarray(a).astype(ml_dtypes.bfloat16)


def make_mask():
    import ml_dtypes
    tk = np.arange(128)[:, None]
    tq = np.arange(128)[None, :]
    return (tq >= tk).astype(ml_dtypes.bfloat16)


def shard_inputs(data, Wq, Wk, Wv, Wp):
    """Build the 8 per-core input maps from full inputs."""
    data = np.asarray(data, np.float32)
    Wq = np.asarray(Wq, np.float32)
    Wk = np.asarray(Wk, np.float32)
    Wv = np.asarray(Wv, np.float32)
    Wp = np.asarray(Wp, np.float32)
    mask = make_mask()
    in_maps = []
    for c in range(N_CORES):
        b, g = c // 2, c % 2
        hs = slice(g * H_LOC, (g + 1) * H_LOC)
        wq = Wq[hs].transpose(1, 0, 2).reshape(E, C)
        wk = Wk[hs].transpose(1, 0, 2).reshape(E, C)
        wv = Wv[hs].transpose(1, 0, 2).reshape(E, C)
        in_maps.append({
            "xT": _bf(data[b].T),
            "wqkv": _bf(np.concatenate([wq, wk, wv], axis=1)),
            "wp": _bf(Wp[g * C:(g + 1) * C, :]),
            "mask": mask,
        })
    return in_maps


_NC_CACHE = {}


def legalize_single_wait(nc):
    """This toolchain's walrus accepts at most ONE sync wait per engine
    instruction; Tile freely emits more. Split extra waits onto preceding
    same-engine NoOps (engine FIFOs make that equivalent)."""
    import bass_rust
    cnt = 0
    for f in nc.m.functions:
        for blk in f.blocks:
            new = []
            changed = False
            for inst in blk.instructions:
                si = inst.sync_info
                if si is not None and len(si.on_wait) > 1:
                    waits = list(si.on_wait)
                    for w in waits[:-1]:
                        nop = bass_rust.InstNoOp(name=f"legal_nop_{cnt}")
                        cnt += 1
                        nop.engine = inst.engine
                        nop.sync_info = bass_rust.SyncInfo(on_wait=[w],
                                                           on_update=[])
                        new.append(nop)
                    inst.sync_info = bass_rust.SyncInfo(
                        on_wait=[waits[-1]], on_update=list(si.on_update))
                    changed = True
                new.append(inst)
            if changed:
                blk.instructions = new
    return cnt


def get_nc():
    if "nc" not in _NC_CACHE:
        nc = bass.Bass("TRN2", target_bir_lowering=False, debug=False,
                       num_devices=N_CORES)
        build_program(nc)
        legalize_single_wait(nc)
        _NC_CACHE["nc"] = nc
    return _NC_CACHE["nc"]


def run(inputs, trace=False, **kw):
    """Run on the 8 NeuronCores; returns (full_output, BassKernelResults)."""
    from concourse.bass_utils import run_bass_kernel_spmd
    nc = get_nc()
    in_maps = shard_inputs(inputs["data"], inputs["Wq"], inputs["Wk"],
                           inputs["Wv"], inputs["Wp"])
    res = run_bass_kernel_spmd(nc, in_maps, core_ids=list(range(N_CORES)),
                               trace=trace, **kw)
    bp = np.asarray(inputs["bp"], np.float32)
    outf = np.empty((B, T, E), np.float32)
    for b in range(B):
        outf[b] = res.results[2 * b]["out"] + res.results[2 * b + 1]["out"] \
            + bp
    return outf, res


def kernel(**inputs):
    out, _ = run(inputs)
    return out


# revision 20
# speedup vs baseline: 1.2298x; 1.0713x over previous
"""Trainium2 Bass kernel for multi-head causal attention + output projection.

Problem (hardcoded): B=4, T=2048, E=1024, H=16, D=64, float32.
  q = einsum('bte,hed->bhtd', data, Wq)   (same k, v)
  scores = q@k.T / sqrt(D), causal mask, softmax
  out = (attn @ v) concat-heads @ Wp + bp

Sharding across 8 NeuronCores: core c -> (batch b=c//2, head-group g=c%2).
Each core computes 8 heads (4 "pairs" of 2) of one batch.

Per-core design (all matmul operands bf16; PSUM f32):
 - q/k projections into qT/kT [d-pair(128) x T]; v into vallp[p]
   [t-in-tile(128) x (t-tile, head, d)] natural layout.
 - scores kept TRANSPOSED (psS[tk, tq]) so the softmax key-reduction is a
   PE contraction; exp on ACT (no max subtraction: scores ~ N(0,1)).
 - attn@V with the EXP TILE AS STATIONARY and V as moving: output lands
   in natural [tq, d] layout at full PE utilization (64-row moving), and
   sum(exp) comes from 1-row ones-matmuls into a shared psum bank.
 - normalization = per-partition reciprocal + tensor_scalar_mul on DVE
   (no PE broadcast needed in natural layout).
 - attention output transposed back to [c, t] via PE transpose for the
   output projection; Wp projection accumulates over 4 c-tiles.
 - causal: key-tiles above the diagonal skipped; diagonal tiles trim the
   query range to >= o*128 and mask only the 128x128 boundary triangle.
 - a filler queue spreads q/k/v chunk projections and the output
   projection into the ACT-bound attention windows so the PE never
   idles waiting for exp.

PSUM bank map (2KB x 8):
  banks 0-3: psS double-buffer (2 x [128,1024] f32)
  banks 4-5: psAV parity     (2 x [128,512] f32: 8 x 64-col accumulators)
  bank  6:   misc: sumexp accumulators (cols 0:16, parity 8) + transpose
             staging (bf16 view of f32 cols 64:320)
  bank  7:   pq: q/k/v chunk projections, then output projection

Host side: out[b] = core(2b) + core(2b+1) partials, + bias.
"""

import numpy as np

import concourse.bass as bass
import concourse.mybir as mybir
import concourse.tile as tile
from concourse import masks as cmasks
from contextlib import ExitStack

F32 = mybir.dt.float32
BF16 = mybir.dt.bfloat16

# Full-problem constants
B, T, E, H, D = 4, 2048, 1024, 16, 64
N_CORES = 8
H_LOC = H // 2          # heads per core
HP = H_LOC // 2         # head pairs per core
C = H_LOC * D           # local concat width (512)
ET = E // 128           # embedding 128-tiles
TT = T // 128           # token 128-tiles
NJB = 4                 # query blocks of 512
TQB = 512
SCALE = float(D) ** -0.5

AV_LAG = 5              # tiles between exp and attn@V consumption


def build_program(nc):
    AF = mybir.ActivationFunctionType

    xTd = nc.dram_tensor("xT", [E, T], BF16, kind="ExternalInput").ap()
    wqkvd = nc.dram_tensor("wqkv", [E, 3 * C], BF16,
                           kind="ExternalInput").ap()
    wpd = nc.dram_tensor("wp", [C, E], BF16, kind="ExternalInput").ap()
    maskd = nc.dram_tensor("mask", [128, 128], BF16,
                           kind="ExternalInput").ap()
    outd = nc.dram_tensor("out", [T, E], F32, kind="ExternalOutput").ap()

    with tile.TileContext(nc) as tc, ExitStack() as ctx:
        sb = ctx.enter_context(tc.tile_pool(name="sb", bufs=1))
        ident = sb.tile([128, 128], BF16, name="ident")
        mask_sb = sb.tile([128, 128], BF16, name="mask_sb")
        wqkvt = [sb.tile([128, 3 * C], BF16, name=f"wqkvt{e}")
                 for e in range(ET)]
        xt = [sb.tile([128, T], BF16, name=f"xt{e}") for e in range(ET)]
        qT = [sb.tile([128, T], BF16, name=f"qT{p}") for p in range(HP)]
        kT = [sb.tile([128, T], BF16, name=f"kT{p}") for p in range(HP)]
        # v in natural layout + ones column: col = (t*8 + lh)*65 + d,
        # d=64 is the ones column (yields sum(exp) through the AV matmul)
        vall = sb.tile([128, TT * H_LOC * 65], BF16, name="vall")
        # attention out, natural [tq x (tt, h, d)] per pair
        olt_nat = [sb.tile([128, T], BF16, name=f"oltn{p}") for p in range(HP)]
        # attention out, transposed [c x t] per pair c-block
        olt = [sb.tile([128, T], BF16, name=f"olt{p}") for p in range(HP)]
        wpt = [sb.tile([128, E], BF16, name=f"wpt{p}") for p in range(HP)]

        ee_pool = ctx.enter_context(tc.tile_pool(name="ee", bufs=AV_LAG + 2))
        rr_pool = ctx.enter_context(tc.tile_pool(name="rr", bufs=4))
        ot_pool = ctx.enter_context(tc.tile_pool(name="ot", bufs=2))

        ps_pool = ctx.enter_context(
            tc.tile_pool(name="ps", bufs=1, space="PSUM"))
        psS_pool = ctx.enter_context(
            tc.tile_pool(name="pss", bufs=2, space="PSUM"))
        # per-head AV accumulators: 4 x 65 cols each, one bank per head
        psAVa = ps_pool.tile([128, 260], F32, name="psAVa")
        psAVb = ps_pool.tile([128, 260], F32, name="psAVb")
        psT = ps_pool.tile([128, 512], F32, name="psT")
        pq = ps_pool.tile([128, 512], F32, name="pqbank")
        psT_bf = psT[:, 0:256].bitcast(BF16)  # [128, 512] bf16 region

        cmasks.make_identity(nc, ident[:])
        nc.gpsimd.memset(vall[:, 64::65], 1.0)  # ones columns
        nc.sync.dma_start(mask_sb[:], maskd)
        for e in range(ET):
            nc.sync.dma_start(wqkvt[e][:], wqkvd[e * 128:(e + 1) * 128, :])
        for e in range(ET):
            nc.sync.dma_start(xt[e][:, 0:512], xTd[e * 128:(e + 1) * 128,
                                                   0:512])
        for cch in range(1, 4):
            for e in range(ET):
                nc.sync.dma_start(
                    xt[e][:, cch * 512:(cch + 1) * 512],
                    xTd[e * 128:(e + 1) * 128, cch * 512:(cch + 1) * 512])
        for p in range(HP):
            nc.sync.dma_start(wpt[p][:], wpd[p * 128:(p + 1) * 128, :])

        # ---------------- filler work-queue ------------------------------
        # Items: (key, fn). key=(p, c) for chunk work block (p, jb>=c)
        # depends on; None for output-projection work (no deadline).
        filler = []

        def emit_v_round(t):
            # all 8 heads of one token-tile: psv [128t x 512(lh,d)]
            for e in range(ET):
                nc.tensor.matmul(
                    pq[:, 0:512],
                    xt[e][:, t * 128:(t + 1) * 128],
                    wqkvt[e][:, 2 * C:3 * C],
                    start=(e == 0), stop=(e == ET - 1))
            # scatter into vall's 65-wide head blocks (ones col untouched)
            src = bass.AP(pq.tensor, pq.offset,
                          [list(pq.ap[0]), [64, 8], [1, 64]])
            dst = bass.AP(vall.tensor, vall.offset + t * 520,
                          [list(vall.ap[0]), [65, 8], [1, 64]])
            nc.vector.tensor_copy(dst, src)

        def emit_qk_part(p, cch, woff, half):
            # half of one 512-wide q or k chunk (4 e-steps)
            for e in range(4 * half, 4 * half + 4):
                nc.tensor.matmul(
                    pq[:, 0:512],
                    wqkvt[e][:, woff + p * 128:woff + (p + 1) * 128],
                    xt[e][:, cch * 512:(cch + 1) * 512],
                    start=(e == 0), stop=(e == ET - 1))

        def emit_qk_evict(dst, p, cch):
            nc.vector.tensor_copy(
                dst[p][:, cch * 512:(cch + 1) * 512], pq[:, 0:512])

        def enqueue_v(cch):
            # V rounds cover ALL pairs; key (0, cch) so they drain before
            # any pair's block cch
            for t in range(4 * cch, 4 * cch + 4):
                filler.append(((0, cch), lambda t=t: emit_v_round(t)))

        def enqueue_qk(p, cch):
            key = (p, cch)
            filler.append((key, lambda: emit_qk_part(p, cch, 0, 0)))
            filler.append((key, lambda: (emit_qk_part(p, cch, 0, 1),
                                         emit_qk_evict(qT, p, cch))))
            filler.append((key, lambda: emit_qk_part(p, cch, C, 0)))
            filler.append((key, lambda: (emit_qk_part(p, cch, C, 1),
                                         emit_qk_evict(kT, p, cch))))

        def drain(n):
            for _ in range(n):
                if not filler:
                    return
                _, fn = filler.pop(0)
                fn()

        def drain_until(p, jb):
            while filler:
                key = filler[0][0]
                if key is not None:
                    kp, kc = key
                    if (kp, kc) > (p, jb):
                        return
                filler.pop(0)[1]()

        # ---------------- output projection work -------------------------
        def emit_transp(tt):
            for p4 in range(HP):
                nc.tensor.transpose(
                    psT_bf[:, p4 * 128:(p4 + 1) * 128],
                    olt_nat[p4][:, tt * 128:(tt + 1) * 128],
                    ident[:])

        def emit_transp_evict(tt):
            # GPSIMD cannot access PSUM; these evictions ride on DVE
            for p4 in range(HP):
                nc.vector.tensor_copy(
                    olt[p4][:, tt * 128:(tt + 1) * 128],
                    psT_bf[:, p4 * 128:(p4 + 1) * 128])

        def emit_proj_mm(tt, ec):
            for p4 in range(HP):
                nc.tensor.matmul(
                    pq[:, 0:512],
                    olt[p4][:, tt * 128:(tt + 1) * 128],
                    wpt[p4][:, ec * 512:(ec + 1) * 512],
                    start=(p4 == 0), stop=(p4 == HP - 1))

        def emit_proj_evict(tt, ec, ot):
            nc.vector.tensor_copy(ot[:, ec * 512:(ec + 1) * 512],
                                  pq[:, 0:512])

        def enqueue_proj(jb):
            for tt in range(4 * jb, 4 * jb + 4):
                ot = ot_pool.tile([128, E], F32, tag="ot", name="ot")

                def fin(tt=tt, ot=ot):
                    emit_proj_evict(tt, 1, ot)
                    nc.sync.dma_start(outd[tt * 128:(tt + 1) * 128, :],
                                      ot[:])
                filler.append((None, lambda tt=tt: emit_transp(tt)))
                filler.append((None, lambda tt=tt: emit_transp_evict(tt)))
                filler.append((None, lambda tt=tt: emit_proj_mm(tt, 0)))
                filler.append((None, lambda tt=tt, ot=ot: (
                    emit_proj_evict(tt, 0, ot), emit_proj_mm(tt, 1))))
                filler.append((None, fin))

        # ---------------- attention --------------------------------------
        def emit_attn_block(p, jb, blk):
            n_tk = 4 * jb + 4
            psAV = (psAVa, psAVb)
            pend = []

            def flush_av():
                # start=True zeroes/marks the WHOLE 2KB psum bank: exactly
                # one start (first matmul of the block into the bank) and
                # one stop (last matmul) per bank per block; the pending-
                # zero mechanism zeroes each 65-col region at first write.
                tk, ee = pend.pop(0)
                o = tk - 4 * jb
                for h in range(2):
                    for s in range(max(0, o), 4):
                        st = ee[:, h * 512 + s * 128:h * 512 + (s + 1) * 128]
                        first = tk == 0 and s == max(0, o)
                        last = tk == n_tk - 1 and s == 3
                        nc.tensor.matmul(
                            psAV[h][:, s * 65:(s + 1) * 65],
                            st,
                            vall[:, (tk * 8 + 2 * p + h) * 65:
                                 (tk * 8 + 2 * p + h) * 65 + 65],
                            start=first, stop=last, skip_group_check=True)

            for tk in range(n_tk):
                o = tk - 4 * jb
                lo = max(0, o) * 128
                ksl = slice(tk * 128, (tk + 1) * 128)
                psS = psS_pool.tile([128, 1024], F32, tag="s", name="psS")
                nc.tensor.matmul(
                    psS[:, lo:512],
                    kT[p][0:64, ksl],
                    qT[p][0:64, jb * TQB + lo:(jb + 1) * TQB],
                    start=True, stop=True)
                nc.tensor.matmul(
                    psS[:, 512 + lo:1024],
                    kT[p][64:128, ksl],
                    qT[p][64:128, jb * TQB + lo:(jb + 1) * TQB],
                    start=True, stop=True)
                ee = ee_pool.tile([128, 1024], BF16, tag="ee", name="ee")
                nc.scalar.activation(ee[:, lo:1024], psS[:, lo:1024],
                                     AF.Exp, scale=SCALE)
                if o >= 0:
                    for h in range(2):
                        r = slice(h * 512 + o * 128, h * 512 + (o + 1) * 128)
                        nc.vector.tensor_mul(ee[:, r], ee[:, r], mask_sb[:])
                pend.append((tk, ee))
                drain(2 if p == HP - 1 else 1)
                if len(pend) > AV_LAG:
                    flush_av()
            while pend:
                flush_av()

            # normalize: 1/sumexp per tq partition (col 64 of each region),
            # scale psAV into olt_nat
            for h in range(2):
                rr = rr_pool.tile([128, 4], F32, tag="rr", name="rr")
                nc.vector.reciprocal(rr[:], psAV[h][:, 64::65])
                for s in range(4):
                    tt = 4 * jb + s
                    nc.vector.tensor_scalar_mul(
                        olt_nat[p][:, tt * 128 + h * 64:tt * 128 +
                                   (h + 1) * 64],
                        psAV[h][:, s * 65:s * 65 + 64],
                        rr[:, s:s + 1])

        # ---------------- schedule ---------------------------------------
        # prefix: V chunk 0 (all pairs) + q/k chunk 0 of pair 0, direct
        for t in range(4):
            emit_v_round(t)
        emit_qk_part(0, 0, 0, 0)
        emit_qk_part(0, 0, 0, 1)
        emit_qk_evict(qT, 0, 0)
        emit_qk_part(0, 0, C, 0)
        emit_qk_part(0, 0, C, 1)
        emit_qk_evict(kT, 0, 0)
        # queue: remaining chunks in need order (keys must be FIFO-ordered)
        for cch in range(1, NJB):
            enqueue_v(cch)
            enqueue_qk(0, cch)
        for p in range(1, HP):
            for cch in range(NJB):
                enqueue_qk(p, cch)

        blk = 0
        for p in range(HP):
            for jb in range(NJB):
                drain_until(p, jb)
                emit_attn_block(p, jb, blk)
                blk += 1
                if p == HP - 1:
                    enqueue_proj(jb)
        drain(len(filler))
    return nc


# ---------------- host side ----------------------------------------------

def _bf(a):
    import ml_dtypes
    return np.ascontiguousarray(a).astype(ml_dtypes.bfloat16)


def make_mask():
    import ml_dtypes
    tk = np.arange(128)[:, None]
    tq = np.arange(128)[None, :]
    return (tq >= tk).astype(ml_dtypes.bfloat16)


def shard_inputs(data, Wq, Wk, Wv, Wp):
    """Build the 8 per-core input maps from full inputs."""
    data = np.asarray(data, np.float32)
    Wq = np.asarray(Wq, np.float32)
    Wk = np.asarray(Wk, np.float32)
    Wv = np.asarray(Wv, np.float32)
    Wp = np.asarray(Wp, np.float32)
    mask = make_mask()
    in_maps = []
    for c in range(N_CORES):
        b, g = c // 2, c % 2
        hs = slice(g * H_LOC, (g + 1) * H_LOC)
        wq = Wq[hs].transpose(1, 0, 2).reshape(E, C)
        wk = Wk[hs].transpose(1, 0, 2).reshape(E, C)
        wv = Wv[hs].transpose(1, 0, 2).reshape(E, C)
        in_maps.append({
            "xT": _bf(data[b].T),
            "wqkv": _bf(np.concatenate([wq, wk, wv], axis=1)),
            "wp": _bf(Wp[g * C:(g + 1) * C, :]),
            "mask": mask,
        })
    return in_maps


_NC_CACHE = {}


def legalize_single_wait(nc):
    """This toolchain's walrus accepts at most ONE sync wait per engine
    instruction; Tile freely emits more. Split extra waits onto preceding
    same-engine NoOps (engine FIFOs make that equivalent)."""
    import bass_rust
    cnt = 0
    for f in nc.m.functions:
        for blk in f.blocks:
            new = []
            changed = False
            for inst in blk.instructions:
                si = inst.sync_info
                if si is not None and len(si.on_wait) > 1:
                    waits = list(si.on_wait)
                    for w in waits[:-1]:
                        nop = bass_rust.InstNoOp(name=f"legal_nop_{cnt}")
                        cnt += 1
                        nop.engine = inst.engine
                        nop.sync_info = bass_rust.SyncInfo(on_wait=[w],
                                                           on_update=[])
                        new.append(nop)
                    inst.sync_info = bass_rust.SyncInfo(
                        on_wait=[waits[-1]], on_update=list(si.on_update))
                    changed = True
                new.append(inst)
            if changed:
                blk.instructions = new
    return cnt


def get_nc():
    if "nc" not in _NC_CACHE:
        nc = bass.Bass("TRN2", target_bir_lowering=False, debug=False,
                       num_devices=N_CORES)
        build_program(nc)
        legalize_single_wait(nc)
        _NC_CACHE["nc"] = nc
    return _NC_CACHE["nc"]


def run(inputs, trace=False, **kw):
    """Run on the 8 NeuronCores; returns (full_output, BassKernelResults)."""
    from concourse.bass_utils import run_bass_kernel_spmd
    nc = get_nc()
    in_maps = shard_inputs(inputs["data"], inputs["Wq"], inputs["Wk"],
                           inputs["Wv"], inputs["Wp"])
    res = run_bass_kernel_spmd(nc, in_maps, core_ids=list(range(N_CORES)),
                               trace=trace, **kw)
    bp = np.asarray(inputs["bp"], np.float32)
    outf = np.empty((B, T, E), np.float32)
    for b in range(B):
        outf[b] = res.results[2 * b]["out"] + res.results[2 * b + 1]["out"] \
            + bp
    return outf, res


def kernel(**inputs):
    out, _ = run(inputs)
    return out
